# revision 20
# baseline (speedup 1.0000x reference)
"""Multi-head attention (B=2, C=256, N=64*64=4096, heads=8, d=32) on 8 trn2
NeuronCores via Bass/Tile.

Sharding: batch x head-pair. Core c handles batch c//4 and heads
(2*(c%4), 2*(c%4)+1); each core computes the full NxN attention for its two
heads plus its partial contribution to the output projection; the host sums
the 4 per-batch partials (each core adds b_out/4 so the sum carries the bias
exactly once).

Structure (ScalarE exp is the wall at ~264us busy; wall ~348us):
  - Q/K projected into 4x-replicated per-head [128, N] f32r tiles; the NxN
    score matmuls row-tile at K=32 across three rotating PE bands.
    KEEP THE BAND ROTATION: with a fixed band the per-matmul weight loads
    serialize, the clock governor (ham k=4/8 windows in the ntff) drops the
    PE to half clock, and the run regresses 25-50%.
  - V is computed transposed ([64, N] f32r matmuls like Q/K, bias on the
    drain) and moved to the [token, d] AV layout with PE transpose matmuls
    against a bf16 identity (saves ~10us of PE vs fp32 V matmuls).
  - Waves of 3 j-blocks: 3 banded score matmuls -> [128, 1536] PSUM slot ->
    one ScalarE exp (scale=16; scale/16 is folded into w_q/b_q on the host)
    -> bf16 ew -> 3 AV matmuls (M=33, ones-column gives the softmax
    denominator) accumulated in a pair-long PSUM bank. Software-pipelined
    exactly like the 354us baseline.
  - QKV projection groups beyond what wave (0,0,0) needs are deferred and
    injected one per wave into the early wave loop; all drains ride the DVE.
  - A custom 2-instruction DVE exp chain (deg-4 Horner for e^t, c1==c0, then
    4 squarings -> e^16t) is implemented and correct, but DVE_EXP_SHARE
    defaults to 0: pushing >~25% of exp waves to the DVE trips the same
    half-clock activity throttle (k4 ~200us) and regresses the run. fp8e4m3
    DoubleRow AV was also evaluated: 2x PE on paper, but fp8 attention
    weights alone cost rel_err 0.022 > the 0.02 gate (peaked softmax rows).
"""

import sys

sys.path.insert(0, "/opt/trn_rl_repo")

import numpy as np

B, C, HW, N = 2, 256, 64, 4096
HEADS, D = 8, 32
SCALE = float(D) ** -0.5
FOLD = SCALE / 16.0     # folded into w_q/b_q on the host
NCORES = 8
IB = 512            # i-block width
N_IB = N // IB      # 8
JB = 128            # j-block height
N_JB = N // JB      # 32
WAVE_JB = 3         # j-blocks per wave
N_WAVE = (N_JB + WAVE_JB - 1) // WAVE_JB  # 11 (10x3 + 1x2)

# deg-4 Horner for e^t on [-0.64, 0.64] with c1 == c0 (fit rel err 1.9e-4;
# after ^16 the weights carry ~3e-3 relative error, below bf16 AV noise).
EXP_C0 = 0.9999375157595985
EXP_C2 = 0.5002585069221668
EXP_C3 = 0.16985172072103063
EXP_C4 = 0.04191900003510396

# per-instruction cost estimates (us) for the ACT/DVE greedy balancer
ACT_NS_PER_COL = 0.000833
ACT_FIXED = 0.21
DVE_NS_PER_COL = 0.0010417
DVE_FIXED = 0.22
# Max fraction of exp waves offloaded to the DVE. The board's activity
# monitor clamps the NC to a 50% utilization limit when total engine
# activity runs too hot (throttle_activity_1 in the ntff summary), so DVE
# offload is NOT free: at 27% share the run spent 73% of its time at half
# clock and regressed. Tune against measured throttle.
DVE_EXP_SHARE = 0.0
import os as _os
DVE_EXP_SHARE = float(_os.environ.get("DVE_EXP_SHARE", DVE_EXP_SHARE))
W10_DVE = _os.environ.get("W10_DVE", "0") == "1"

_COMPILED = None
_OPS_REGISTERED = False


def _register_dve_ops():
    """Register the 2-instruction exp chain as custom DVE ops (idempotent)."""
    global _OPS_REGISTERED
    if _OPS_REGISTERED:
        return
    import concourse.dve_ops as dv
    from concourse.dve_spec import (
        C0, C1, C2, C3, Spec, Src0, _spill_c3_to_src1, lower, sq,
    )
    from concourse.dve_uop import DveOpSpec

    if "EXP16P1_ANT" in dv._SUB_OPCODE_FOR_NAME:
        _OPS_REGISTERED = True
        return

    t = Src0
    # (((c4 t + c3) t + c2) t + c0) t + c0   [c3->C0, c2->C1, c0->C2, c4->C3]
    body1 = _spill_c3_to_src1((((C3 * t + C0) * t + C1) * t + C2) * t + C2)

    def _ref_p1(in0, in1, s0, s1, imm2):
        tt = in0.astype(np.float32)
        return (((((in1 * tt + s0) * tt + s1) * tt + imm2) * tt) + imm2).astype(
            np.float32
        )

    spec1 = Spec(body=body1, reference=_ref_p1)

    body2 = sq(sq(sq(sq(Src0))))

    def _ref_p2(in0, in1, s0, s1, imm2):
        x = in0.astype(np.float32)
        x = x * x
        x = x * x
        x = x * x
        return (x * x).astype(np.float32)

    spec2 = Spec(body=body2, reference=_ref_p2)

    next_row = max(dv._SUB_OPCODE_FOR_NAME.values()) + 1
    for name, spec in (("EXP16P1_ANT", spec1), ("EXP16SQ4_ANT", spec2)):
        dv._SUB_OPCODE_FOR_NAME[name] = next_row
        shas = {}
        for ver in ("v3", "v4"):
            uops = lower(spec, ver=ver)
            shas[ver] = DveOpSpec(
                name=name, opcode=next_row, uops=uops,
                rd1_en=(name == "EXP16P1_ANT"),
            ).sha(ver)
        op = dv.DveOp(name, spec, subdim=False, uops_sha=shas)
        dv.OPS.append(op)
        dv.CUSTOM_DVE_SPECS[name] = spec
        next_row += 1
    _OPS_REGISTERED = True


def _build():
    _register_dve_ops()
    from concourse import bacc, mybir
    from concourse.tile import TileContext
    import concourse.dve_ops as dv

    exp_p1 = next(o for o in dv.OPS if o.name == "EXP16P1_ANT")
    exp_p2 = next(o for o in dv.OPS if o.name == "EXP16SQ4_ANT")

    dt = mybir.dt
    f32, f32r, bf16 = dt.float32, dt.float32r, dt.bfloat16
    fp16 = dt.float16
    Exp = mybir.ActivationFunctionType.Exp
    MULT = mybir.AluOpType.mult

    nc = bacc.Bacc("TRN2", target_bir_lowering=False, debug=False,
                   enable_asserts=True, num_devices=NCORES)

    xb = nc.dram_tensor("xb", [C, N], bf16, kind="ExternalInput").ap()
    wq_pk = nc.dram_tensor("wq_pk", [2, C, 128], bf16, kind="ExternalInput").ap()
    wk_pk = nc.dram_tensor("wk_pk", [2, C, 128], bf16, kind="ExternalInput").ap()
    wv_pk = nc.dram_tensor("wv_pk", [C, 2 * D], bf16, kind="ExternalInput").ap()
    bq_pk = nc.dram_tensor("bq_pk", [2, 128], f32, kind="ExternalInput").ap()
    bk_pk = nc.dram_tensor("bk_pk", [2, 128], f32, kind="ExternalInput").ap()
    bv_pk = nc.dram_tensor("bv_pk", [2 * D], f32, kind="ExternalInput").ap()
    wo = nc.dram_tensor("wo", [D, 2 * C], bf16, kind="ExternalInput").ap()
    bo4 = nc.dram_tensor("bo4", [C], f32, kind="ExternalInput").ap()
    ident = nc.dram_tensor("ident", [2 * D, 2 * D], bf16, kind="ExternalInput").ap()
    o = nc.dram_tensor("o", [C, N], f32, kind="ExternalOutput").ap()

    with TileContext(nc) as tc:
        with tc.tile_pool(name="const", bufs=1) as cp, \
             tc.tile_pool(name="big", bufs=1) as bp, \
             tc.tile_pool(name="work", bufs=2) as wkp, \
             tc.tile_pool(name="ewp", bufs=4) as ep, \
             tc.tile_pool(name="scr", bufs=2) as sp, \
             tc.tile_pool(name="ps", bufs=2, space="PSUM") as ps:

            # ---- x -> SBUF in 1024-col chunks, already bf16 on the host
            # (halves DMA bytes, no device-side rounding casts).  Chunk 0 of
            # both C-halves lands first, then the critical h0/bias weights
            # ride each queue, then the x tail. -----------------------------
            XCH = 1024
            x_r = [bp.tile([128, N], bf16, name=f"x_r{cc}", tag=f"x_r{cc}")
                   for cc in range(2)]

            def x_chunk(cc, ch):
                cs = slice(ch * XCH, (ch + 1) * XCH)
                eng = (nc.sync, nc.scalar)[cc]
                eng.dma_start(out=x_r[cc][:, cs],
                              in_=xb[128 * cc:128 * (cc + 1), cs])

            x_chunk(0, 0)
            x_chunk(1, 0)

            def load_w(name, src, shape, eng):
                w = cp.tile(shape, bf16, name=name + "_r", tag=name + "_r")
                eng.dma_start(out=w[:], in_=src)
                return w

            # critical path weights: K/V/Q head 0 + biases, split across the
            # two HWDGE queues right behind x chunk 0
            wk_r = [[None, None], [None, None]]
            wq_r = [[None, None], [None, None]]
            for cc in range(2):
                wk_r[0][cc] = load_w(f"wk0{cc}", wk_pk[0, 128 * cc:128 * (cc + 1), :],
                                     [128, 128], nc.sync)
            wv_r = [load_w(f"wv{cc}", wv_pk[128 * cc:128 * (cc + 1), :], [128, 2 * D],
                           nc.sync) for cc in range(2)]
            for cc in range(2):
                wq_r[0][cc] = load_w(f"wq0{cc}", wq_pk[0, 128 * cc:128 * (cc + 1), :],
                                     [128, 128], nc.scalar)
            bq_sb = cp.tile([128, 2], f32, name="bq_sb", tag="bq_sb")
            bk_sb = cp.tile([128, 2], f32, name="bk_sb", tag="bk_sb")
            for h in range(2):
                nc.scalar.dma_start(out=bq_sb[:, h:h + 1], in_=bq_pk[h, :])
                nc.scalar.dma_start(out=bk_sb[:, h:h + 1], in_=bk_pk[h, :])
            bv_sb = cp.tile([2 * D, 1], f32, name="bv_sb", tag="bv_sb")
            nc.scalar.dma_start(out=bv_sb[:, 0:1], in_=bv_pk[:])
            id_bf = cp.tile([2 * D, 2 * D], bf16, name="id_bf", tag="id_bf")
            nc.scalar.dma_start(out=id_bf[:], in_=ident[:, :])

            # x tail
            for ch in range(1, N // XCH):
                x_chunk(0, ch)
                x_chunk(1, ch)

            # non-critical weights on the gpsimd (SWDGE) queue: head-1
            # projections + output projection + its bias
            for cc in range(2):
                wk_r[1][cc] = load_w(f"wk1{cc}", wk_pk[1, 128 * cc:128 * (cc + 1), :],
                                     [128, 128], nc.gpsimd)
                wq_r[1][cc] = load_w(f"wq1{cc}", wq_pk[1, 128 * cc:128 * (cc + 1), :],
                                     [128, 128], nc.gpsimd)
            wo_r = cp.tile([D, 2 * C], bf16, name="wo_r", tag="wo_r")
            nc.gpsimd.dma_start(out=wo_r[:], in_=wo[:, :])
            bo_sb = cp.tile([128, 2], f32, name="bo_sb", tag="bo_sb")
            for cc in range(2):
                nc.gpsimd.dma_start(out=bo_sb[:, cc:cc + 1], in_=bo4[128 * cc:128 * (cc + 1)])

            # [P,1] tile carrying the poly's 4th coefficient (C3 spill)
            c4_sb = cp.tile([128, 1], f32, name="c4_sb", tag="c4_sb")
            nc.vector.memset(c4_sb[:], EXP_C4)

            # ---- PE warm-up during the x DMAs: dense back-to-back matmuls
            # so the HAM sees a fully-busy window and lifts the PE clock
            # before the first real projection ------------------------------
            warm_t = cp.tile([128, 512], bf16, name="warm_t", tag="warm_t")
            nc.vector.memset(warm_t[:], 1.0)
            wslot = ps.tile([128, 512], f32, tag="av", name="warm_ps")
            for _ in range(5):
                nc.tensor.matmul(out=wslot[:, :], lhsT=warm_t[:, 0:128], rhs=warm_t[:, :],
                                 start=True, stop=True)

            # ---- QKV projections (packed [64, N]) -------------------------
            qt = [bp.tile([128, N], bf16, name=f"qt{h}", tag=f"qt{h}") for h in range(2)]
            kt = [bp.tile([128, N], bf16, name=f"kt{h}", tag=f"kt{h}") for h in range(2)]
            vt_sb = bp.tile([2 * D, N], bf16, name="vt_sb", tag="vt_sb")

            def qk_group(dst, w_r, b_sb, h, blks, drain_eng):
                slot = ps.tile([128, len(blks) * IB], f32, tag="ps3", name="qk_ps")
                for bi, ib in enumerate(blks):
                    for cc in range(2):
                        nc.tensor.matmul(
                            out=slot[:, bi * IB:(bi + 1) * IB],
                            lhsT=w_r[h][cc][:, :],
                            rhs=x_r[cc][:, ib * IB:(ib + 1) * IB],
                            start=(cc == 0), stop=(cc == 1))
                dsl = dst[h][:, blks[0] * IB:(blks[0] + len(blks)) * IB]
                if drain_eng is nc.scalar:
                    nc.scalar.add(out=dsl, in_=slot[:, :], add=b_sb[:, h:h + 1])
                else:
                    drain_eng.tensor_scalar_add(out=dsl, in0=slot[:, :],
                                                scalar1=b_sb[:, h:h + 1])

            def proj_group(dst, w_r, b_sb, blks, drain_eng, nparts=4 * D):
                slot = ps.tile([nparts, len(blks) * IB], f32, tag="ps3", name="qk_ps")
                for bi, ib in enumerate(blks):
                    for cc in range(2):
                        nc.tensor.matmul(
                            out=slot[:, bi * IB:(bi + 1) * IB],
                            lhsT=w_r[cc][:, :],
                            rhs=x_r[cc][:, ib * IB:(ib + 1) * IB],
                            start=(cc == 0), stop=(cc == 1))
                dsl = dst[:, blks[0] * IB:(blks[0] + len(blks)) * IB]
                if drain_eng is nc.scalar:
                    nc.scalar.add(out=dsl, in_=slot[:, :], add=b_sb[:, 0:1])
                else:
                    drain_eng.tensor_scalar_add(out=dsl, in0=slot[:, :],
                                                scalar1=b_sb[:, 0:1])

            v_sb = bp.tile([128, N_JB, 66], bf16, name="v_sb", tag="v_sb")
            nc.vector.memset(v_sb[:, :, 32:33], 1.0)
            nc.vector.memset(v_sb[:, :, 65:66], 1.0)

            def v_block(blks):
                # V^T proj + drain for `blks`, then transpose each 512-token
                # block (4 j-blocks) into the [token, d] AV layout.
                proj_group(vt_sb, wv_r, bv_sb, blks, nc.vector, nparts=2 * D)
                for g in blks:
                    vt_ps = ps.tile([128, 4, 2 * D], bf16, tag="av", name="vt_ps")
                    for j in range(4):
                        jb = g * 4 + j
                        nc.tensor.matmul(
                            out=vt_ps[:, j, :],
                            lhsT=vt_sb[:, jb * JB:(jb + 1) * JB],
                            rhs=id_bf[:, :],
                            is_transpose=True)
                    nc.vector.tensor_copy(out=v_sb[:, 4 * g:4 * g + 4, 0:32],
                                          in_=vt_ps[:, :, 0:32])
                    nc.vector.tensor_copy(out=v_sb[:, 4 * g:4 * g + 4, 33:65],
                                          in_=vt_ps[:, :, 32:64])

            # minimal pre-wave prolog: only what pair (0,0) wave 0 needs
            # (x chunk 0 only)
            qk_group(kt, wk_r, bk_sb, 0, [0, 1], nc.vector)
            qk_group(qt, wq_r, bq_sb, 0, [0], nc.vector)
            v_block([0, 1])

            # remaining x rounds + projection groups, injected on a
            # need-driven wave schedule (urgent K/V early behind their x
            # chunks, relaxed Q groups into pairs 2-5).
            deferred = [
                lambda: qk_group(kt, wk_r, bk_sb, 0, [2, 3], nc.vector),
                lambda: v_block([2, 3]),
                lambda: qk_group(kt, wk_r, bk_sb, 0, [4, 5], nc.vector),
                lambda: v_block([4, 5]),
                lambda: qk_group(kt, wk_r, bk_sb, 0, [6, 7], nc.vector),
                lambda: v_block([6, 7]),
                lambda: qk_group(kt, wk_r, bk_sb, 1, [0, 1, 2], nc.vector),
                lambda: qk_group(qt, wq_r, bq_sb, 1, [0, 1, 2], nc.vector),
                lambda: qk_group(kt, wk_r, bk_sb, 1, [3, 4, 5], nc.vector),
                lambda: qk_group(kt, wk_r, bk_sb, 1, [6, 7], nc.vector),
                lambda: qk_group(qt, wq_r, bq_sb, 0, [1, 2], nc.vector),
                lambda: qk_group(qt, wq_r, bq_sb, 0, [3, 4, 5], nc.vector),
                lambda: qk_group(qt, wq_r, bq_sb, 1, [3, 4, 5], nc.vector),
                lambda: qk_group(qt, wq_r, bq_sb, 0, [6, 7], nc.vector),
                lambda: qk_group(qt, wq_r, bq_sb, 1, [6, 7], nc.vector),
            ]
            inject_at = [1, 2, 4, 5, 7, 8, 9, 10, 12, 14, 16, 24, 35, 46, 57]
            gw_state = [0]

            # ---- attention waves + projection -----------------------------
            pairs = []
            for n in range(N_IB):
                pairs.append((n, 0))
                pairs.append((n, 1))

            pending = []

            def flush_pending():
                if pending:
                    pending.pop(0)()

            accs = {}
            eng_t = {"act": 0.0, "dve": 0.0}   # projected finish (us)

            def emit_av(pend):
                ew_p, jbs_p, av_p, n_p, h_p = pend
                for r, jb in enumerate(jbs_p):
                    nc.tensor.matmul(
                        out=av_p[0:33, :],
                        lhsT=v_sb[:, jb, 33 * h_p:33 * h_p + 33],
                        rhs=ew_p[:, r * IB:(r + 1) * IB],
                        start=(jb == 0), stop=(jb == N_JB - 1),
                        tile_position=(0, 0))
                if jbs_p[-1] != N_JB - 1:
                    return
                acc = accs[n_p]
                nc.vector.tensor_copy(out=acc[:, h_p * IB:(h_p + 1) * IB],
                                      in_=av_p[0:33, :])
                eng_t["dve"] += 0.6
                if h_p == 0:
                    return
                recip = wkp.tile([33, 1024], f32, tag="recip", name="recip", bufs=1)
                nc.vector.tensor_copy(out=recip[0:1, :], in_=acc[32:33, :])
                recipf = wkp.tile([1, 1024], f32, tag="recipf", name="recipf", bufs=1)
                nc.vector.reciprocal_approx_fast(out=recipf[0:1, :], in_=recip[0:1, :])
                eng_t["dve"] += 1.3
                bc_sb = wkp.tile([32, 1024], f32, tag="bc_sb", name="bc_sb", bufs=1)
                nc.gpsimd.partition_broadcast(bc_sb[:, :], recipf[0:1, :])

                norm = wkp.tile([32, 1024], bf16, tag="norm", name="norm", bufs=1)

                def phase3_cc(cc, n=n_p, acc=acc, bc_sb=bc_sb, norm=norm):
                    i0 = n * IB
                    if cc == 0:
                        nc.vector.tensor_tensor(out=norm[:, :], in0=bc_sb[:, :],
                                                in1=acc[0:32, :], op=MULT)
                        eng_t["dve"] += 1.2
                    pj = ps.tile([128, IB], f32, tag="av", name="pj_ps")
                    for hh in range(2):
                        nc.tensor.matmul(
                            out=pj[:, :],
                            lhsT=wo_r[:, hh * C + cc * 128: hh * C + (cc + 1) * 128],
                            rhs=norm[:, hh * IB:(hh + 1) * IB],
                            start=(hh == 0), stop=(hh == 1), tile_position=(0, 0))
                    y = wkp.tile([128, IB], f32, tag="y", name="y")
                    nc.vector.tensor_scalar_add(out=y[:], in0=pj[:, :],
                                                scalar1=bo_sb[:, cc:cc + 1])
                    eng_t["dve"] += 0.65
                    nc.sync.dma_start(out=o[cc * 128:(cc + 1) * 128, i0:i0 + IB],
                                      in_=y[:])

                pending.append(lambda: phase3_cc(0))
                pending.append(lambda: phase3_cc(1))

            pend = None  # (ew, jbs, av, n, h) of the wave awaiting its AV
            for n, h in pairs:
                i0 = n * IB
                if h == 0:
                    accs[n] = wkp.tile([33, 1024], f32, tag="acc", name="acc", bufs=3)
                av = ps.tile([128, IB], f32, tag="av", name="av_ps")

                for w in range(N_WAVE):
                    jbs = list(range(w * WAVE_JB, min((w + 1) * WAVE_JB, N_JB)))
                    cols = len(jbs) * IB
                    slot = ps.tile([128, cols], f32, tag="ps3", name="wave_ps")
                    for r, jb in enumerate(jbs):
                        nc.tensor.matmul(
                            out=slot[:, r * IB:(r + 1) * IB],
                            lhsT=kt[h][32 * r:32 * r + 32, jb * JB:(jb + 1) * JB],
                            rhs=qt[h][32 * r:32 * r + 32, i0:i0 + IB],
                            start=True, stop=True, tile_position=(32 * r, 0))
                    if pend is not None:
                        emit_av(pend)
                    if w in (4, 7):
                        flush_pending()
                    ew = ep.tile([128, cols], bf16, tag="ew", name="ew")
                    act_cost = cols * ACT_NS_PER_COL + ACT_FIXED
                    dve_cost = 2 * cols * DVE_NS_PER_COL + 2 * DVE_FIXED
                    nw = eng_t["nwave"] = eng_t.get("nwave", 0) + 1
                    dve_ok = eng_t.get("ndve", 0) < DVE_EXP_SHARE * nw
                    if (w == 8 and W10_DVE) or (
                            dve_ok and eng_t["dve"] + dve_cost < eng_t["act"] + act_cost):
                        eng_t["ndve"] = eng_t.get("ndve", 0) + 1
                        scr = sp.tile([128, cols], fp16, tag="scr", name="scr", bufs=1)
                        nc.vector._custom_dve(
                            exp_p1, out=scr[:, :], in0=slot[:, :], in1=c4_sb[:, :],
                            s0=EXP_C3, s1=EXP_C2, imm2=EXP_C0)
                        nc.vector._custom_dve(exp_p2, out=ew[:], in0=scr[:, :])
                        eng_t["dve"] += dve_cost
                    else:
                        nc.scalar.activation(out=ew[:], in_=slot[:, :], func=Exp,
                                             scale=16.0)
                        eng_t["act"] += act_cost
                    pend = (ew, jbs, av, n, h)
                    gw_state[0] += 1
                    while deferred and inject_at and gw_state[0] >= inject_at[0]:
                        inject_at.pop(0)
                        deferred.pop(0)()
            emit_av(pend)
            while pending:
                flush_pending()

    nc.finalize()
    return nc


def _get_compiled():
    global _COMPILED
    if _COMPILED is None:
        _COMPILED = _build()
    return _COMPILED


def _make_in_maps(x, w_qkv, b_qkv, w_out, b_out):
    x = np.asarray(x, dtype=np.float32)
    w_qkv = np.asarray(w_qkv, dtype=np.float32)
    b_qkv = np.asarray(b_qkv, dtype=np.float32)
    w_out = np.asarray(w_out, dtype=np.float32)
    b_out = np.asarray(b_out, dtype=np.float32)

    import ml_dtypes
    bf = ml_dtypes.bfloat16

    xf = x.reshape(B, C, N)
    xf_bf = [np.ascontiguousarray(xf[b].astype(bf)) for b in range(B)]
    ident = np.eye(2 * D, dtype=bf)
    in_maps = []
    for core in range(NCORES):
        b = core // 4
        q = core % 4
        heads = (2 * q, 2 * q + 1)

        def rep_cols(w_slice):
            return np.ascontiguousarray(np.tile(w_slice, (1, 4)))

        wq = np.stack([rep_cols(w_qkv[:, 32 * h:32 * h + 32] * FOLD) for h in heads])
        wk = np.stack([rep_cols(w_qkv[:, C + 32 * h:C + 32 * h + 32]) for h in heads])
        wv = np.concatenate([w_qkv[:, 2 * C + 32 * h:2 * C + 32 * h + 32] for h in heads], axis=1)
        bq = np.stack([np.tile(b_qkv[32 * h:32 * h + 32] * FOLD, 4) for h in heads])
        bk = np.stack([np.tile(b_qkv[C + 32 * h:C + 32 * h + 32], 4) for h in heads])
        bv = np.concatenate([b_qkv[2 * C + 32 * h:2 * C + 32 * h + 32] for h in heads])
        wo_ = np.concatenate([w_out[32 * h:32 * h + 32, :] for h in heads], axis=1)
        in_maps.append({
            "xb": xf_bf[b],
            "wq_pk": np.ascontiguousarray(wq.astype(bf)),
            "wk_pk": np.ascontiguousarray(wk.astype(bf)),
            "wv_pk": np.ascontiguousarray(wv.astype(bf)),
            "bq_pk": np.ascontiguousarray(bq),
            "bk_pk": np.ascontiguousarray(bk),
            "bv_pk": np.ascontiguousarray(bv),
            "wo": np.ascontiguousarray(wo_.astype(bf)),
            "bo4": np.ascontiguousarray(b_out / 4.0),
            "ident": ident,
        })
    return in_maps


def kernel(x, w_qkv, b_qkv, w_out, b_out, _trace=False, _trace_kwargs=None):
    from concourse.bass_utils import run_bass_kernel_spmd

    nc = _get_compiled()
    in_maps = _make_in_maps(x, w_qkv, b_qkv, w_out, b_out)
    res = run_bass_kernel_spmd(nc, in_maps, list(range(NCORES)),
                               trace=_trace, **(_trace_kwargs or {}))
    parts = [res.results[c]["o"] for c in range(NCORES)]
    out = np.empty((B, C, N), dtype=np.float32)
    for b in range(B):
        out[b] = parts[4 * b] + parts[4 * b + 1] + parts[4 * b + 2] + parts[4 * b + 3]
    result = out.reshape(B, C, HW, HW)
    if _trace:
        return result, res
    return result



# revision 22
# speedup vs baseline: 1.2243x; 1.2243x over previous
"""Multi-head attention (B=2, C=256, N=64*64=4096, heads=8, d=32) on 8 trn2
NeuronCores via Bass/Tile.

Sharding: batch x head-pair. Core c handles batch c//4 and heads
(2*(c%4), 2*(c%4)+1); each core computes the full NxN attention for its two
heads plus its partial contribution to the output projection; the host sums
the 4 per-batch partials (each core adds b_out/4 so the sum carries the bias
exactly once).

Structure (ScalarE exp is the wall at ~264us busy; wall ~348us):
  - Q/K projected into 4x-replicated per-head [128, N] f32r tiles; the NxN
    score matmuls row-tile at K=32 across three rotating PE bands.
    KEEP THE BAND ROTATION: with a fixed band the per-matmul weight loads
    serialize, the clock governor (ham k=4/8 windows in the ntff) drops the
    PE to half clock, and the run regresses 25-50%.
  - V is computed transposed ([64, N] f32r matmuls like Q/K, bias on the
    drain) and moved to the [token, d] AV layout with PE transpose matmuls
    against a bf16 identity (saves ~10us of PE vs fp32 V matmuls).
  - Waves of 3 j-blocks: 3 banded score matmuls -> [128, 1536] PSUM slot ->
    one ScalarE exp (scale=16; scale/16 is folded into w_q/b_q on the host)
    -> bf16 ew -> 3 AV matmuls (M=33, ones-column gives the softmax
    denominator) accumulated in a pair-long PSUM bank. Software-pipelined
    exactly like the 354us baseline.
  - QKV projection groups beyond what wave (0,0,0) needs are deferred and
    injected one per wave into the early wave loop; all drains ride the DVE.
  - A custom 2-instruction DVE exp chain (deg-4 Horner for e^t, c1==c0, then
    4 squarings -> e^16t) is implemented and correct, but DVE_EXP_SHARE
    defaults to 0: pushing >~25% of exp waves to the DVE trips the same
    half-clock activity throttle (k4 ~200us) and regresses the run. fp8e4m3
    DoubleRow AV was also evaluated: 2x PE on paper, but fp8 attention
    weights alone cost rel_err 0.022 > the 0.02 gate (peaked softmax rows).
"""

import sys

sys.path.insert(0, "/opt/trn_rl_repo")

import numpy as np

B, C, HW, N = 2, 256, 64, 4096
HEADS, D = 8, 32
SCALE = float(D) ** -0.5
FOLD = SCALE / 16.0     # folded into w_q/b_q on the host
NCORES = 8
IB = 512            # i-block width
N_IB = N // IB      # 8
JB = 128            # j-block height
N_JB = N // JB      # 32
WAVE_JB = 3         # j-blocks per wave
N_WAVE = (N_JB + WAVE_JB - 1) // WAVE_JB  # 11 (10x3 + 1x2)

# deg-4 Horner for e^t on [-0.64, 0.64] with c1 == c0 (fit rel err 1.9e-4;
# after ^16 the weights carry ~3e-3 relative error, below bf16 AV noise).
EXP_C0 = 0.9999375157595985
EXP_C2 = 0.5002585069221668
EXP_C3 = 0.16985172072103063
EXP_C4 = 0.04191900003510396

# per-instruction cost estimates (us) for the ACT/DVE greedy balancer
ACT_NS_PER_COL = 0.000833
ACT_FIXED = 0.21
DVE_NS_PER_COL = 0.0010417
DVE_FIXED = 0.22
# Max fraction of exp waves offloaded to the DVE. The board's activity
# monitor clamps the NC to a 50% utilization limit when total engine
# activity runs too hot (throttle_activity_1 in the ntff summary), so DVE
# offload is NOT free: at 27% share the run spent 73% of its time at half
# clock and regressed. Tune against measured throttle.
DVE_EXP_SHARE = 0.0
import os as _os
DVE_EXP_SHARE = float(_os.environ.get("DVE_EXP_SHARE", DVE_EXP_SHARE))
W10_DVE = _os.environ.get("W10_DVE", "0") == "1"

_COMPILED = None
_OPS_REGISTERED = False


def _register_dve_ops():
    """Register the 2-instruction exp chain as custom DVE ops (idempotent)."""
    global _OPS_REGISTERED
    if _OPS_REGISTERED:
        return
    import concourse.dve_ops as dv
    from concourse.dve_spec import (
        C0, C1, C2, C3, Spec, Src0, _spill_c3_to_src1, lower, sq,
    )
    from concourse.dve_uop import DveOpSpec

    if "EXP16P1_ANT" in dv._SUB_OPCODE_FOR_NAME:
        _OPS_REGISTERED = True
        return

    t = Src0
    # (((c4 t + c3) t + c2) t + c0) t + c0   [c3->C0, c2->C1, c0->C2, c4->C3]
    body1 = _spill_c3_to_src1((((C3 * t + C0) * t + C1) * t + C2) * t + C2)

    def _ref_p1(in0, in1, s0, s1, imm2):
        tt = in0.astype(np.float32)
        return (((((in1 * tt + s0) * tt + s1) * tt + imm2) * tt) + imm2).astype(
            np.float32
        )

    spec1 = Spec(body=body1, reference=_ref_p1)

    body2 = sq(sq(sq(sq(Src0))))

    def _ref_p2(in0, in1, s0, s1, imm2):
        x = in0.astype(np.float32)
        x = x * x
        x = x * x
        x = x * x
        return (x * x).astype(np.float32)

    spec2 = Spec(body=body2, reference=_ref_p2)

    next_row = max(dv._SUB_OPCODE_FOR_NAME.values()) + 1
    for name, spec in (("EXP16P1_ANT", spec1), ("EXP16SQ4_ANT", spec2)):
        dv._SUB_OPCODE_FOR_NAME[name] = next_row
        shas = {}
        for ver in ("v3", "v4"):
            uops = lower(spec, ver=ver)
            shas[ver] = DveOpSpec(
                name=name, opcode=next_row, uops=uops,
                rd1_en=(name == "EXP16P1_ANT"),
            ).sha(ver)
        op = dv.DveOp(name, spec, subdim=False, uops_sha=shas)
        dv.OPS.append(op)
        dv.CUSTOM_DVE_SPECS[name] = spec
        next_row += 1
    _OPS_REGISTERED = True


def _build():
    _register_dve_ops()
    from concourse import bacc, mybir
    from concourse.tile import TileContext
    import concourse.dve_ops as dv

    exp_p1 = next(o for o in dv.OPS if o.name == "EXP16P1_ANT")
    exp_p2 = next(o for o in dv.OPS if o.name == "EXP16SQ4_ANT")

    dt = mybir.dt
    f32, f32r, bf16 = dt.float32, dt.float32r, dt.bfloat16
    fp16 = dt.float16
    Exp = mybir.ActivationFunctionType.Exp
    MULT = mybir.AluOpType.mult

    nc = bacc.Bacc("TRN2", target_bir_lowering=False, debug=False,
                   enable_asserts=True, num_devices=NCORES)

    xb = nc.dram_tensor("xb", [C, N], bf16, kind="ExternalInput").ap()
    wq_pk = nc.dram_tensor("wq_pk", [2, C, 128], bf16, kind="ExternalInput").ap()
    wk_pk = nc.dram_tensor("wk_pk", [2, C, 128], bf16, kind="ExternalInput").ap()
    wv_pk = nc.dram_tensor("wv_pk", [C, 2 * D], bf16, kind="ExternalInput").ap()
    bq_pk = nc.dram_tensor("bq_pk", [2, 128], f32, kind="ExternalInput").ap()
    bk_pk = nc.dram_tensor("bk_pk", [2, 128], f32, kind="ExternalInput").ap()
    bv_pk = nc.dram_tensor("bv_pk", [2 * D], f32, kind="ExternalInput").ap()
    wo = nc.dram_tensor("wo", [D, 2 * C], bf16, kind="ExternalInput").ap()
    bo4 = nc.dram_tensor("bo4", [C], f32, kind="ExternalInput").ap()
    ident = nc.dram_tensor("ident", [2 * D, 2 * D], bf16, kind="ExternalInput").ap()
    o = nc.dram_tensor("o", [C, N], f32, kind="ExternalOutput").ap()

    with TileContext(nc) as tc:
        with tc.tile_pool(name="const", bufs=1) as cp, \
             tc.tile_pool(name="big", bufs=1) as bp, \
             tc.tile_pool(name="work", bufs=2) as wkp, \
             tc.tile_pool(name="ewp", bufs=4) as ep, \
             tc.tile_pool(name="scr", bufs=2) as sp, \
             tc.tile_pool(name="ps", bufs=2, space="PSUM") as ps:

            # ---- x -> SBUF in 1024-col chunks, already bf16 on the host
            # (halves DMA bytes, no device-side rounding casts).  Chunk 0 of
            # both C-halves lands first, then the critical h0/bias weights
            # ride each queue, then the x tail. -----------------------------
            XCH = 1024
            x_r = [bp.tile([128, N], bf16, name=f"x_r{cc}", tag=f"x_r{cc}")
                   for cc in range(2)]

            def x_chunk(cc, ch):
                cs = slice(ch * XCH, (ch + 1) * XCH)
                eng = (nc.sync, nc.scalar)[cc]
                eng.dma_start(out=x_r[cc][:, cs],
                              in_=xb[128 * cc:128 * (cc + 1), cs])

            x_chunk(0, 0)
            x_chunk(1, 0)

            def load_w(name, src, shape, eng):
                w = cp.tile(shape, bf16, name=name + "_r", tag=name + "_r")
                eng.dma_start(out=w[:], in_=src)
                return w

            # critical path weights: K/V/Q head 0 + biases, split across the
            # two HWDGE queues right behind x chunk 0
            wk_r = [[None, None], [None, None]]
            wq_r = [[None, None], [None, None]]
            for cc in range(2):
                wk_r[0][cc] = load_w(f"wk0{cc}", wk_pk[0, 128 * cc:128 * (cc + 1), :],
                                     [128, 128], nc.sync)
            wv_r = [load_w(f"wv{cc}", wv_pk[128 * cc:128 * (cc + 1), :], [128, 2 * D],
                           nc.sync) for cc in range(2)]
            for cc in range(2):
                wq_r[0][cc] = load_w(f"wq0{cc}", wq_pk[0, 128 * cc:128 * (cc + 1), :],
                                     [128, 128], nc.scalar)
            bq_sb = cp.tile([128, 2], f32, name="bq_sb", tag="bq_sb")
            bk_sb = cp.tile([128, 2], f32, name="bk_sb", tag="bk_sb")
            for h in range(2):
                nc.scalar.dma_start(out=bq_sb[:, h:h + 1], in_=bq_pk[h, :])
                nc.scalar.dma_start(out=bk_sb[:, h:h + 1], in_=bk_pk[h, :])
            bv_sb = cp.tile([2 * D, 1], f32, name="bv_sb", tag="bv_sb")
            nc.scalar.dma_start(out=bv_sb[:, 0:1], in_=bv_pk[:])
            id_bf = cp.tile([2 * D, 2 * D], bf16, name="id_bf", tag="id_bf")
            nc.scalar.dma_start(out=id_bf[:], in_=ident[:, :])

            # x tail
            for ch in range(1, N // XCH):
                x_chunk(0, ch)
                x_chunk(1, ch)

            # non-critical weights on the gpsimd (SWDGE) queue: head-1
            # projections + output projection + its bias
            for cc in range(2):
                wk_r[1][cc] = load_w(f"wk1{cc}", wk_pk[1, 128 * cc:128 * (cc + 1), :],
                                     [128, 128], nc.gpsimd)
                wq_r[1][cc] = load_w(f"wq1{cc}", wq_pk[1, 128 * cc:128 * (cc + 1), :],
                                     [128, 128], nc.gpsimd)
            wo_r = cp.tile([D, 2 * C], bf16, name="wo_r", tag="wo_r")
            nc.gpsimd.dma_start(out=wo_r[:], in_=wo[:, :])
            bo_sb = cp.tile([128, 2], f32, name="bo_sb", tag="bo_sb")
            for cc in range(2):
                nc.gpsimd.dma_start(out=bo_sb[:, cc:cc + 1], in_=bo4[128 * cc:128 * (cc + 1)])

            # [P,1] tile carrying the poly's 4th coefficient (C3 spill)
            c4_sb = cp.tile([128, 1], f32, name="c4_sb", tag="c4_sb")
            nc.vector.memset(c4_sb[:], EXP_C4)

            # ---- PE warm-up during the x DMAs: dense back-to-back matmuls
            # so the HAM sees a fully-busy window and lifts the PE clock
            # before the first real projection ------------------------------
            warm_t = cp.tile([128, 512], bf16, name="warm_t", tag="warm_t")
            nc.vector.memset(warm_t[:], 1.0)
            wslot = ps.tile([128, 512], f32, tag="av", name="warm_ps")
            for _ in range(5):
                nc.tensor.matmul(out=wslot[:, :], lhsT=warm_t[:, 0:128], rhs=warm_t[:, :],
                                 start=True, stop=True)

            # ---- QKV projections (packed [64, N]) -------------------------
            qt = [bp.tile([128, N], bf16, name=f"qt{h}", tag=f"qt{h}") for h in range(2)]
            kt = [bp.tile([128, N], bf16, name=f"kt{h}", tag=f"kt{h}") for h in range(2)]
            vt_sb = bp.tile([2 * D, N], bf16, name="vt_sb", tag="vt_sb")

            def qk_group(dst, w_r, b_sb, h, blks, drain_eng):
                slot = ps.tile([128, len(blks) * IB], f32, tag="ps3", name="qk_ps")
                for bi, ib in enumerate(blks):
                    for cc in range(2):
                        nc.tensor.matmul(
                            out=slot[:, bi * IB:(bi + 1) * IB],
                            lhsT=w_r[h][cc][:, :],
                            rhs=x_r[cc][:, ib * IB:(ib + 1) * IB],
                            start=(cc == 0), stop=(cc == 1))
                dsl = dst[h][:, blks[0] * IB:(blks[0] + len(blks)) * IB]
                if drain_eng is nc.scalar:
                    nc.scalar.add(out=dsl, in_=slot[:, :], add=b_sb[:, h:h + 1])
                else:
                    drain_eng.tensor_scalar_add(out=dsl, in0=slot[:, :],
                                                scalar1=b_sb[:, h:h + 1])

            def proj_group(dst, w_r, b_sb, blks, drain_eng, nparts=4 * D):
                slot = ps.tile([nparts, len(blks) * IB], f32, tag="ps3", name="qk_ps")
                for bi, ib in enumerate(blks):
                    for cc in range(2):
                        nc.tensor.matmul(
                            out=slot[:, bi * IB:(bi + 1) * IB],
                            lhsT=w_r[cc][:, :],
                            rhs=x_r[cc][:, ib * IB:(ib + 1) * IB],
                            start=(cc == 0), stop=(cc == 1))
                dsl = dst[:, blks[0] * IB:(blks[0] + len(blks)) * IB]
                if drain_eng is nc.scalar:
                    nc.scalar.add(out=dsl, in_=slot[:, :], add=b_sb[:, 0:1])
                else:
                    drain_eng.tensor_scalar_add(out=dsl, in0=slot[:, :],
                                                scalar1=b_sb[:, 0:1])

            v_sb = bp.tile([128, N_JB, 66], bf16, name="v_sb", tag="v_sb")
            nc.vector.memset(v_sb[:, :, 32:33], 1.0)
            nc.vector.memset(v_sb[:, :, 65:66], 1.0)

            def v_block(blks):
                # V^T proj + drain for `blks`, then transpose each 512-token
                # block (4 j-blocks) into the [token, d] AV layout.
                proj_group(vt_sb, wv_r, bv_sb, blks, nc.vector, nparts=2 * D)
                for g in blks:
                    vt_ps = ps.tile([128, 4, 2 * D], bf16, tag="av", name="vt_ps")
                    for j in range(4):
                        jb = g * 4 + j
                        nc.tensor.matmul(
                            out=vt_ps[:, j, :],
                            lhsT=vt_sb[:, jb * JB:(jb + 1) * JB],
                            rhs=id_bf[:, :],
                            is_transpose=True)
                    nc.vector.tensor_copy(out=v_sb[:, 4 * g:4 * g + 4, 0:32],
                                          in_=vt_ps[:, :, 0:32])
                    nc.vector.tensor_copy(out=v_sb[:, 4 * g:4 * g + 4, 33:65],
                                          in_=vt_ps[:, :, 32:64])

            # minimal pre-wave prolog: only what pair (0,0) wave 0 needs
            # (x chunk 0 only)
            qk_group(kt, wk_r, bk_sb, 0, [0, 1], nc.vector)
            qk_group(qt, wq_r, bq_sb, 0, [0], nc.vector)
            v_block([0, 1])

            # remaining x rounds + projection groups, injected on a
            # need-driven wave schedule (urgent K/V early behind their x
            # chunks, relaxed Q groups into pairs 2-5).
            deferred = [
                lambda: qk_group(kt, wk_r, bk_sb, 0, [2, 3], nc.vector),
                lambda: v_block([2, 3]),
                lambda: qk_group(kt, wk_r, bk_sb, 0, [4, 5], nc.vector),
                lambda: v_block([4, 5]),
                lambda: qk_group(kt, wk_r, bk_sb, 0, [6, 7], nc.vector),
                lambda: v_block([6, 7]),
                lambda: qk_group(kt, wk_r, bk_sb, 1, [0, 1, 2], nc.vector),
                lambda: qk_group(qt, wq_r, bq_sb, 1, [0, 1, 2], nc.vector),
                lambda: qk_group(kt, wk_r, bk_sb, 1, [3, 4, 5], nc.vector),
                lambda: qk_group(kt, wk_r, bk_sb, 1, [6, 7], nc.vector),
                lambda: qk_group(qt, wq_r, bq_sb, 0, [1, 2], nc.vector),
                lambda: qk_group(qt, wq_r, bq_sb, 0, [3, 4, 5], nc.vector),
                lambda: qk_group(qt, wq_r, bq_sb, 1, [3, 4, 5], nc.vector),
                lambda: qk_group(qt, wq_r, bq_sb, 0, [6, 7], nc.vector),
                lambda: qk_group(qt, wq_r, bq_sb, 1, [6, 7], nc.vector),
            ]
            inject_at = [1, 2, 4, 5, 7, 8, 9, 10, 12, 14, 16, 24, 35, 46, 57]
            gw_state = [0]

            # ---- attention waves + projection -----------------------------
            pairs = []
            for n in range(N_IB):
                pairs.append((n, 0))
                pairs.append((n, 1))

            pending = []

            def flush_pending():
                if pending:
                    pending.pop(0)()

            accs = {}
            eng_t = {"act": 0.0, "dve": 0.0}   # projected finish (us)

            def emit_av(pend):
                ew_p, jbs_p, av_p, n_p, h_p = pend
                for r, jb in enumerate(jbs_p):
                    nc.tensor.matmul(
                        out=av_p[0:33, :],
                        lhsT=v_sb[:, jb, 33 * h_p:33 * h_p + 33],
                        rhs=ew_p[:, r * IB:(r + 1) * IB],
                        start=(jb == 0), stop=(jb == N_JB - 1),
                        tile_position=(0, 0))
                if jbs_p[-1] != N_JB - 1:
                    return
                acc = accs[n_p]
                nc.vector.tensor_copy(out=acc[:, h_p * IB:(h_p + 1) * IB],
                                      in_=av_p[0:33, :])
                eng_t["dve"] += 0.6
                if h_p == 0:
                    return
                recip = wkp.tile([33, 1024], f32, tag="recip", name="recip", bufs=1)
                nc.vector.tensor_copy(out=recip[0:1, :], in_=acc[32:33, :])
                recipf = wkp.tile([1, 1024], f32, tag="recipf", name="recipf", bufs=1)
                nc.vector.reciprocal_approx_fast(out=recipf[0:1, :], in_=recip[0:1, :])
                eng_t["dve"] += 1.3
                bc_sb = wkp.tile([32, 1024], f32, tag="bc_sb", name="bc_sb", bufs=1)
                nc.gpsimd.partition_broadcast(bc_sb[:, :], recipf[0:1, :])

                norm = wkp.tile([32, 1024], bf16, tag="norm", name="norm", bufs=1)

                def phase3_cc(cc, n=n_p, acc=acc, bc_sb=bc_sb, norm=norm):
                    i0 = n * IB
                    if cc == 0:
                        nc.vector.tensor_tensor(out=norm[:, :], in0=bc_sb[:, :],
                                                in1=acc[0:32, :], op=MULT)
                        eng_t["dve"] += 1.2
                    pj = ps.tile([128, IB], f32, tag="av", name="pj_ps")
                    for hh in range(2):
                        nc.tensor.matmul(
                            out=pj[:, :],
                            lhsT=wo_r[:, hh * C + cc * 128: hh * C + (cc + 1) * 128],
                            rhs=norm[:, hh * IB:(hh + 1) * IB],
                            start=(hh == 0), stop=(hh == 1), tile_position=(0, 0))
                    y = wkp.tile([128, IB], f32, tag="y", name="y")
                    nc.vector.tensor_scalar_add(out=y[:], in0=pj[:, :],
                                                scalar1=bo_sb[:, cc:cc + 1])
                    eng_t["dve"] += 0.65
                    nc.sync.dma_start(out=o[cc * 128:(cc + 1) * 128, i0:i0 + IB],
                                      in_=y[:])

                pending.append(lambda: phase3_cc(0))
                pending.append(lambda: phase3_cc(1))

            pend = None  # (ew, jbs, av, n, h) of the wave awaiting its AV
            for n, h in pairs:
                i0 = n * IB
                if h == 0:
                    accs[n] = wkp.tile([33, 1024], f32, tag="acc", name="acc", bufs=3)
                av = ps.tile([128, IB], f32, tag="av", name="av_ps")

                for w in range(N_WAVE):
                    jbs = list(range(w * WAVE_JB, min((w + 1) * WAVE_JB, N_JB)))
                    cols = len(jbs) * IB
                    slot = ps.tile([128, cols], f32, tag="ps3", name="wave_ps")
                    for r, jb in enumerate(jbs):
                        nc.tensor.matmul(
                            out=slot[:, r * IB:(r + 1) * IB],
                            lhsT=kt[h][32 * r:32 * r + 32, jb * JB:(jb + 1) * JB],
                            rhs=qt[h][32 * r:32 * r + 32, i0:i0 + IB],
                            start=True, stop=True, tile_position=(32 * r, 0))
                    if pend is not None:
                        emit_av(pend)
                    if w in (4, 7):
                        flush_pending()
                    ew = ep.tile([128, cols], bf16, tag="ew", name="ew")
                    # Split-wave exp: ACT takes the leading j-blocks, the DVE
                    # custom exp chain takes the trailing 512-col j-block of
                    # some waves.  Whole-wave DVE offload stalls the AV/score
                    # pipeline ~3.4us and trips the PE HAM down to half
                    # clock; a 512-col tail (~1.4us on DVE, overlapped with
                    # the ACT part's AV) does not.
                    eng_t["cols"] = eng_t.get("cols", 0) + cols
                    dve_ok = (cols >= 1024 and
                              eng_t.get("dcols", 0) < DVE_EXP_SHARE * eng_t["cols"])
                    dve_cost = 2 * IB * DVE_NS_PER_COL + 2 * DVE_FIXED
                    if dve_ok and eng_t["dve"] + dve_cost < eng_t["act"] + 2.0:
                        c0 = cols - IB
                        eng_t["dcols"] = eng_t.get("dcols", 0) + IB
                        nc.scalar.activation(out=ew[:, 0:c0], in_=slot[:, 0:c0],
                                             func=Exp, scale=16.0)
                        scr = sp.tile([128, IB], fp16, tag="scr", name="scr", bufs=2)
                        nc.vector._custom_dve(
                            exp_p1, out=scr[:, :], in0=slot[:, c0:cols],
                            in1=c4_sb[:, :], s0=EXP_C3, s1=EXP_C2, imm2=EXP_C0)
                        nc.vector._custom_dve(exp_p2, out=ew[:, c0:cols],
                                              in0=scr[:, :])
                        eng_t["dve"] += dve_cost
                        eng_t["act"] += c0 * ACT_NS_PER_COL + ACT_FIXED
                    else:
                        nc.scalar.activation(out=ew[:], in_=slot[:, :], func=Exp,
                                             scale=16.0)
                        eng_t["act"] += cols * ACT_NS_PER_COL + ACT_FIXED
                    pend = (ew, jbs, av, n, h)
                    gw_state[0] += 1
                    while deferred and inject_at and gw_state[0] >= inject_at[0]:
                        inject_at.pop(0)
                        deferred.pop(0)()
            emit_av(pend)
            while pending:
                flush_pending()

    nc.finalize()
    return nc


def _get_compiled():
    global _COMPILED
    if _COMPILED is None:
        _COMPILED = _build()
    return _COMPILED


def _make_in_maps(x, w_qkv, b_qkv, w_out, b_out):
    x = np.asarray(x, dtype=np.float32)
    w_qkv = np.asarray(w_qkv, dtype=np.float32)
    b_qkv = np.asarray(b_qkv, dtype=np.float32)
    w_out = np.asarray(w_out, dtype=np.float32)
    b_out = np.asarray(b_out, dtype=np.float32)

    import ml_dtypes
    bf = ml_dtypes.bfloat16

    xf = x.reshape(B, C, N)
    xf_bf = [np.ascontiguousarray(xf[b].astype(bf)) for b in range(B)]
    ident = np.eye(2 * D, dtype=bf)
    in_maps = []
    for core in range(NCORES):
        b = core // 4
        q = core % 4
        heads = (2 * q, 2 * q + 1)

        def rep_cols(w_slice):
            return np.ascontiguousarray(np.tile(w_slice, (1, 4)))

        wq = np.stack([rep_cols(w_qkv[:, 32 * h:32 * h + 32] * FOLD) for h in heads])
        wk = np.stack([rep_cols(w_qkv[:, C + 32 * h:C + 32 * h + 32]) for h in heads])
        wv = np.concatenate([w_qkv[:, 2 * C + 32 * h:2 * C + 32 * h + 32] for h in heads], axis=1)
        bq = np.stack([np.tile(b_qkv[32 * h:32 * h + 32] * FOLD, 4) for h in heads])
        bk = np.stack([np.tile(b_qkv[C + 32 * h:C + 32 * h + 32], 4) for h in heads])
        bv = np.concatenate([b_qkv[2 * C + 32 * h:2 * C + 32 * h + 32] for h in heads])
        wo_ = np.concatenate([w_out[32 * h:32 * h + 32, :] for h in heads], axis=1)
        in_maps.append({
            "xb": xf_bf[b],
            "wq_pk": np.ascontiguousarray(wq.astype(bf)),
            "wk_pk": np.ascontiguousarray(wk.astype(bf)),
            "wv_pk": np.ascontiguousarray(wv.astype(bf)),
            "bq_pk": np.ascontiguousarray(bq),
            "bk_pk": np.ascontiguousarray(bk),
            "bv_pk": np.ascontiguousarray(bv),
            "wo": np.ascontiguousarray(wo_.astype(bf)),
            "bo4": np.ascontiguousarray(b_out / 4.0),
            "ident": ident,
        })
    return in_maps


def kernel(x, w_qkv, b_qkv, w_out, b_out, _trace=False, _trace_kwargs=None):
    from concourse.bass_utils import run_bass_kernel_spmd

    nc = _get_compiled()
    in_maps = _make_in_maps(x, w_qkv, b_qkv, w_out, b_out)
    res = run_bass_kernel_spmd(nc, in_maps, list(range(NCORES)),
                               trace=_trace, **(_trace_kwargs or {}))
    parts = [res.results[c]["o"] for c in range(NCORES)]
    out = np.empty((B, C, N), dtype=np.float32)
    for b in range(B):
        out[b] = parts[4 * b] + parts[4 * b + 1] + parts[4 * b + 2] + parts[4 * b + 3]
    result = out.reshape(B, C, HW, HW)
    if _trace:
        return result, res
    return result

